# revision 1
# baseline (speedup 1.0000x reference)
"""Causal multi-head attention (B=2, S=2048, D=1024, H=16, Dh=64) on 8 TRN2 cores.

Sharding: core c -> batch b=c//4, head-group g=c%4 (heads 4g..4g+3, d_out cols
g*256..(g+1)*256). Each core computes Q/K/V projections for its head group from
x[b] and runs causal attention for its 4 heads independently. No collectives.

Per-core dataflow:
  phase A: load x[b]^T (pre-transposed on host) + W slices; PE computes
           Q^T,K^T (f32, head-pair layout [128, S]) and V+ones (fp16,
           [S, 4*65] interleaved per head).
  phase B: per (q-tile, head): S row chunks = Q_h^T.T @ K_h^T into PSUM (f32,
           two heads packed into PE row-groups 0-63 / 64-127), additive causal
           mask on the diagonal block (DVE), row-max (DVE reduce, negated),
           exp(S - max) on ACT (per-partition bias) -> P row fp16, batched
           128-block transpose via DMA xbar into per-(head, q-chunk) k-major
           tiles, then O^T[65, 512] = sum_kt V~[kt].T @ P^T[kt] on PE (fp16,
           row 64 = softmax denominator via the ones column).
  host:    out = (O^T[:64] / O^T[64]) transposed back, assembled across cores.
"""

import math

import numpy as np

B = 2
SEQ = 2048
DIN = 1024
H = 16
DH = 64
NCORES = 8
DO = 256  # d_out columns per core (4 heads)
HPC = 4  # heads per core
KT_N = DIN // 128  # 8 contraction tiles
ST_N = SEQ // 128  # 16 seq tiles
QC_N = SEQ // 512  # 4 q-chunks for PV
NEG = -1.0e9
SUB = 1024  # S-row PSUM subtile length (2 banks)

_CACHE = {}
LAST_RESULTS = None


def _emit_core_kernel(tc, outs, ins):
    from concourse import mybir

    nc = tc.nc
    f32 = mybir.dt.float32
    f16 = mybir.dt.float16
    (outT,) = outs  # [HPC, 65, SEQ] f32
    xT, wq, wk, wv, mask = ins

    from contextlib import ExitStack

    with ExitStack() as ctx:
        consts = ctx.enter_context(tc.tile_pool(name="consts", bufs=1))
        proj_out = ctx.enter_context(tc.tile_pool(name="proj_out", bufs=1))
        xs_pool = ctx.enter_context(tc.tile_pool(name="xs", bufs=2))
        prow_pool = ctx.enter_context(tc.tile_pool(name="prow", bufs=4))
        ptrow_pool = ctx.enter_context(tc.tile_pool(name="ptrow", bufs=2))
        stats = ctx.enter_context(tc.tile_pool(name="stats", bufs=8))
        outp = ctx.enter_context(tc.tile_pool(name="outp", bufs=3))
        ps_main = ctx.enter_context(
            tc.tile_pool(name="ps_main", bufs=3, space="PSUM")
        )
        ps_o = ctx.enter_context(tc.tile_pool(name="ps_o", bufs=2, space="PSUM"))

        mask_sb = consts.tile([128, 128], f32, tag="mask")
        nc.sync.dma_start(mask_sb[:], mask[:])
        w_sb = {}
        for wname, wap in (("wq", wq), ("wk", wk), ("wv", wv)):
            t = consts.tile([128, KT_N, DO], f32, tag=wname, name=f"{wname}_sb")
            nc.sync.dma_start(t[:], wap.rearrange("(k p) n -> p k n", p=128))
            w_sb[wname] = t

        qt_sb = [
            proj_out.tile([128, SEQ], f32, tag=f"qt{m}", name=f"qt{m}")
            for m in range(2)
        ]
        kt_sb = [
            proj_out.tile([128, SEQ], f32, tag=f"kt{m}", name=f"kt{m}")
            for m in range(2)
        ]
        v_sb = [
            proj_out.tile([128, HPC * (DH + 1)], f16, tag=f"v{s}", name=f"v{s}")
            for s in range(ST_N)
        ]

        def emit_proj_group(sc):
            """Project Q^T/K^T s-chunk sc and V rows, streaming x^T chunk."""
            xs = []
            for k in range(KT_N):
                t = xs_pool.tile([128, 512], f32, tag=f"xs{k}", name=f"xs{k}")
                nc.sync.dma_start(
                    t[:], xT[k * 128 : (k + 1) * 128, sc * 512 : (sc + 1) * 512]
                )
                xs.append(t)
            for wname, dst in (("wk", kt_sb), ("wq", qt_sb)):
                for m in range(2):
                    pst = ps_main.tile([128, 512], f32, tag="srow", name="pproj")
                    for k in range(KT_N):
                        nc.tensor.matmul(
                            pst[:],
                            w_sb[wname][:, k, m * 128 : (m + 1) * 128],
                            xs[k][:],
                            start=(k == 0),
                            stop=(k == KT_N - 1),
                        )
                    nc.vector.tensor_copy(
                        dst[m][:, sc * 512 : (sc + 1) * 512], pst[:]
                    )
            for j in range(4):
                st = 4 * sc + j
                psv = ps_main.tile([128, DO], f32, tag="srow", name="pv")
                for k in range(KT_N):
                    nc.tensor.matmul(
                        psv[:],
                        xs[k][:, j * 128 : (j + 1) * 128],
                        w_sb["wv"][:, k, :],
                        start=(k == 0),
                        stop=(k == KT_N - 1),
                    )
                for h in range(HPC):
                    nc.scalar.copy(
                        v_sb[st][:, h * (DH + 1) : h * (DH + 1) + DH],
                        psv[:, h * DH : (h + 1) * DH],
                    )
                    nc.gpsimd.memset(
                        v_sb[st][:, h * (DH + 1) + DH : (h + 1) * (DH + 1)], 1.0
                    )

        def emit_attention_qc(qc):
            """Rows qt in qc*4..qc*4+3 for all heads, then PV for the chunk."""
            pt_tiles = {}
            for h in range(HPC):
                pt_tiles[h] = ptrow_pool.tile(
                    [128, ST_N, 512], f16, tag=f"pt{h % 2}", name=f"pt{h % 2}"
                )
            for qt in range(qc * 4, qc * 4 + 4):
                L = (qt + 1) * 128
                for h in range(HPC):
                    m2, poff = h // 2, (h % 2) * 64
                    lhsT_q = qt_sb[m2][poff : poff + 64, qt * 128 : (qt + 1) * 128]
                    subs = [(0, min(L, SUB))]
                    if L > SUB:
                        subs.append((SUB, L - SUB))
                    mneg_parts = stats.tile([128, 2], f32, tag="mneg_p", name="mneg_p")
                    ps_tiles = []
                    for si, (off, ls) in enumerate(subs):
                        ps = ps_main.tile([128, SUB], f32, tag="srow", name="srow")
                        ps_tiles.append((ps, off, ls))
                        for c0 in range(0, ls, 512):
                            c1 = min(ls, c0 + 512)
                            nc.tensor.matmul(
                                ps[:, c0:c1],
                                lhsT_q,
                                kt_sb[m2][poff : poff + 64, off + c0 : off + c1],
                                start=True,
                                stop=True,
                            )
                        if off + ls == L:
                            nc.vector.tensor_add(
                                ps[:, ls - 128 : ls], ps[:, ls - 128 : ls], mask_sb[:]
                            )
                        nc.vector.reduce_max(
                            mneg_parts[:, si : si + 1],
                            ps[:, :ls],
                            axis=mybir.AxisListType.X,
                            negate=True,
                        )
                    if len(subs) == 2:
                        mneg = stats.tile([128, 1], f32, tag="mneg", name="mneg")
                        nc.vector.tensor_reduce(
                            mneg[:, 0:1],
                            mneg_parts[:, 0:2],
                            axis=mybir.AxisListType.X,
                            op=mybir.AluOpType.min,
                        )
                        mneg_ap = mneg[:, 0:1]
                    else:
                        mneg_ap = mneg_parts[:, 0:1]

                    p_row = prow_pool.tile([128, SEQ], f16, tag="prow", name="prow")
                    for ps, off, ls in ps_tiles:
                        nc.scalar.activation(
                            p_row[:, off : off + ls],
                            ps[:, :ls],
                            mybir.ActivationFunctionType.Exp,
                            bias=mneg_ap,
                            scale=1.0,
                        )
                    eng = nc.sync if h % 2 == 0 else nc.scalar
                    eng.dma_start_transpose(
                        pt_tiles[h][
                            :, : qt + 1, (qt % 4) * 128 : (qt % 4) * 128 + 128
                        ],
                        p_row[:, :L],
                    )
            for h in range(HPC):
                po = ps_o.tile([65, 512], f32, tag="po", name="po")
                kt_hi = qc * 4 + 3
                for kt in range(kt_hi + 1):
                    off = max(0, (kt - qc * 4)) * 128
                    nc.tensor.matmul(
                        po[:, off:512],
                        v_sb[kt][:, h * (DH + 1) : (h + 1) * (DH + 1)],
                        pt_tiles[h][:, kt, off:512],
                        start=(kt == 0),
                        stop=(kt == kt_hi),
                    )
                ot = outp.tile([65, 512], f32, tag="ot", name="ot")
                nc.vector.tensor_copy(ot[:], po[:])
                nc.sync.dma_start(outT[h, :, qc * 512 : (qc + 1) * 512], ot[:])

        # merged emission: proj chunk sc, then attention q-chunk sc
        for sc in range(SEQ // 512):
            emit_proj_group(sc)
            emit_attention_qc(sc)


def _split_waits(nc):
    """This container's walrus accepts at most ONE sync-wait per instruction
    on several opcodes ("Too many sync wait commands"). Hoist excess waits
    into standalone InstEventSemaphore instructions on the same engine."""
    from concourse import mybir

    cap = 1
    n = 0
    for f in nc.m.functions:
        for bb in f.blocks:
            new = []
            for inst in list(bb.instructions):
                si = inst.sync_info
                waits = list(si.on_wait) if si is not None else []
                if len(waits) > cap:
                    for j, w in enumerate(waits[cap:]):
                        new.append(
                            mybir.InstEventSemaphore(
                                name=f"{inst.name}-w{j}",
                                engine=inst.engine,
                                ins=[],
                                outs=[],
                                sync_info=mybir.SyncInfo(on_wait=[w], on_update=[]),
                            )
                        )
                        n += 1
                    inst.sync_info = mybir.SyncInfo(
                        on_wait=waits[:cap], on_update=list(si.on_update)
                    )
                new.append(inst)
            bb.instructions = new
    return n


def _build_nc():
    import concourse.bass as bass
    import concourse.tile as tile
    from concourse import mybir

    f32 = mybir.dt.float32
    nc = bass.Bass(
        "TRN2",
        target_bir_lowering=False,
        debug=False,
        num_devices=NCORES,
    )
    xT = nc.dram_tensor("xT", [DIN, SEQ], f32, kind="ExternalInput").ap()
    wq = nc.dram_tensor("wq", [DIN, DO], f32, kind="ExternalInput").ap()
    wk = nc.dram_tensor("wk", [DIN, DO], f32, kind="ExternalInput").ap()
    wv = nc.dram_tensor("wv", [DIN, DO], f32, kind="ExternalInput").ap()
    mask = nc.dram_tensor("mask", [128, 128], f32, kind="ExternalInput").ap()
    outT = nc.dram_tensor("outT", [HPC, DH + 1, SEQ], f32, kind="ExternalOutput").ap()

    with tile.TileContext(nc) as tc:
        _emit_core_kernel(tc, (outT,), (xT, wq, wk, wv, mask))
    _split_waits(nc)
    return nc


def make_mask():
    m = np.zeros((128, 128), dtype=np.float32)
    q = np.arange(128)[:, None]
    k = np.arange(128)[None, :]
    m[k > q] = NEG
    return m


def shard_inputs(x, W_q, W_k, W_v):
    x = np.asarray(x, dtype=np.float32)
    W_q = np.asarray(W_q, dtype=np.float32)
    W_k = np.asarray(W_k, dtype=np.float32)
    W_v = np.asarray(W_v, dtype=np.float32)
    mask = make_mask()
    scale = 1.0 / math.sqrt(DH)
    in_maps = []
    for c in range(NCORES):
        b, g = divmod(c, NCORES // B)
        sl = slice(g * DO, (g + 1) * DO)
        in_maps.append(
            {
                "xT": np.ascontiguousarray(x[b].T),
                "wq": np.ascontiguousarray(W_q[:, sl] * scale),
                "wk": np.ascontiguousarray(W_k[:, sl]),
                "wv": np.ascontiguousarray(W_v[:, sl]),
                "mask": mask,
            }
        )
    return in_maps


def assemble_output(results):
    out = np.zeros((B, SEQ, DIN), dtype=np.float32)
    for c in range(NCORES):
        b, g = divmod(c, NCORES // B)
        oT = results[c]["outT"]  # [HPC, 65, SEQ]
        for h in range(HPC):
            col = g * DO + h * DH
            out[b, :, col : col + DH] = (oT[h, :DH, :] / oT[h, DH : DH + 1, :]).T
    return out


def _install_axon_ntff_hook():
    """Provide antenv.axon_hooks (missing in this image) so trace=True works
    under axon. Mirrors trn_agent_boot.trn_boot._ntff_profile_via_ctypes."""
    import contextlib
    import ctypes
    import sys
    import types

    if "antenv.axon_hooks" in sys.modules:
        return True
    try:
        lib = ctypes.CDLL("/opt/axon/libaxon_pjrt.so")
    except OSError:
        return False
    if not hasattr(lib, "axon_start_nrt_profile"):
        return False
    lib.axon_start_nrt_profile.argtypes = [
        ctypes.POINTER(ctypes.c_int64),
        ctypes.c_size_t,
    ]
    lib.axon_start_nrt_profile.restype = ctypes.c_int64
    lib.axon_stop_nrt_profile.argtypes = [ctypes.c_char_p]
    lib.axon_stop_nrt_profile.restype = ctypes.c_int64

    @contextlib.contextmanager
    def _hook(output_dir, device_ids):
        import jax

        jax.devices()
        if device_ids:
            ids = (ctypes.c_int64 * len(device_ids))(*device_ids)
            rc = lib.axon_start_nrt_profile(ids, len(device_ids))
        else:
            rc = lib.axon_start_nrt_profile(None, 0)
        if rc != 0:
            raise RuntimeError(f"axon_start_nrt_profile rc={rc}")
        try:
            yield
        finally:
            n = lib.axon_stop_nrt_profile(str(output_dir).encode())
            print(f"ntff profile: {n} file(s) written to {output_dir}")

    mod = types.ModuleType("antenv.axon_hooks")
    holder = [_hook]
    mod.get_axon_ntff_profile_hook = lambda: holder[0]
    mod.set_axon_ntff_profile_hook = lambda h: holder.__setitem__(0, h)
    sys.modules["antenv.axon_hooks"] = mod
    import antenv

    antenv.axon_hooks = mod
    return True


def kernel(x, W_q, W_k, W_v):
    global LAST_RESULTS
    import os

    import concourse.bass_utils as bass_utils
    from concourse.bass_utils import run_bass_kernel_spmd

    if "nc" not in _CACHE:
        _CACHE["nc"] = _build_nc()
    nc = _CACHE["nc"]

    in_maps = shard_inputs(x, W_q, W_k, W_v)

    trace = bool(int(os.environ.get("MHA_TRACE", "0")))
    if trace:
        trace = _install_axon_ntff_hook()
        # avoid the fish-bucket artifact upload in this container
        bass_utils.upload_artifacts = lambda d: str(d)
    res = run_bass_kernel_spmd(
        nc, in_maps, core_ids=list(range(NCORES)), trace=trace
    )
    LAST_RESULTS = res
    return assemble_output(res.results)



# revision 17
# speedup vs baseline: 1.1176x; 1.1176x over previous
"""Causal multi-head attention (B=2, S=2048, D=1024, H=16, Dh=64) on 8 TRN2 cores.

Sharding: core c -> batch b=c//4, head-group g=c%4 (heads 4g..4g+3, d_out cols
g*256..(g+1)*256). Each core computes Q/K/V projections for its head group from
x[b] and runs causal attention for its 4 heads independently. No collectives.

All matmuls run in fp16 (1 cyc/col vs fp32's ~4.2). Precision on the logit
path (std ~1024, near-one-hot softmax => argmax flips dominate error) is kept
via hi/lo fp16 splits:
  proj:   x = xhi+xlo, W = Whi+Wlo (split on host); q = xhi*Whi + xlo*Whi +
          xhi*Wlo (3 passes, drops xlo*Wlo ~ 2^-23 rel).
  scores: q = qhi+qlo, k = khi+klo (split on device from proj psum);
          s = qhi*khi + qlo*khi + qhi*klo. K=64 slots for heads h,h+1 run
          CONCURRENTLY via PE row tiling tile_position=(0,0)/(64,0).
Softmax avoids holding full score rows in PSUM: each 512-col chunk is reduced
(chunk max, negated) and shifted (s - m_c) into a fp16 row in SBUF; after all
chunks the global max is combined and exp runs in-place on ACT with per-chunk
bias (m_c - m). V carries a ones column so PV's matmul also yields the softmax
denominator; host divides and assembles.
"""

import math

import numpy as np

B = 2
SEQ = 2048
DIN = 1024
H = 16
DH = 64
NCORES = 8
DO = 256  # d_out columns per core (4 heads)
HPC = 4  # heads per core
KT_N = DIN // 128  # 8 contraction tiles
ST_N = SEQ // 128  # 16 seq tiles
QC_N = SEQ // 512  # 4 q-chunks
NEG = -30000.0  # stays finite in f16 after (s - m_c) shift; exp -> 0

_CACHE = {}
LAST_RESULTS = None


def _emit_core_kernel(tc, outs, ins):
    from concourse import mybir

    nc = tc.nc
    f32 = mybir.dt.float32
    f16 = mybir.dt.float16
    (outT,) = outs  # [HPC, 65, SEQ] f32
    xhi, xlo, wqhi, wqlo, wkhi, wklo, wv, mask = ins

    from contextlib import ExitStack

    with ExitStack() as ctx:
        consts = ctx.enter_context(tc.tile_pool(name="consts", bufs=1))
        proj_out = ctx.enter_context(tc.tile_pool(name="proj_out", bufs=1))
        xs_pool = ctx.enter_context(tc.tile_pool(name="xs", bufs=1))
        scr_pool = ctx.enter_context(tc.tile_pool(name="scr", bufs=1))
        prow_pool = ctx.enter_context(tc.tile_pool(name="prow", bufs=1))
        ptrow_pool = ctx.enter_context(tc.tile_pool(name="ptrow", bufs=2))
        stats = ctx.enter_context(tc.tile_pool(name="stats", bufs=8))
        outp = ctx.enter_context(tc.tile_pool(name="outp", bufs=2))
        ps_sc = ctx.enter_context(
            tc.tile_pool(name="ps_sc", bufs=3, space="PSUM")
        )
        ps_o = ctx.enter_context(tc.tile_pool(name="ps_o", bufs=2, space="PSUM"))

        mask_sb = consts.tile([128, 128], f32, tag="mask", name="mask")
        nc.sync.dma_start(mask_sb[:], mask[:])
        w_sb = {}
        for wname, wap in (
            ("wqhi", wqhi),
            ("wqlo", wqlo),
            ("wkhi", wkhi),
            ("wklo", wklo),
            ("wv", wv),
        ):
            t = consts.tile([128, KT_N, DO], f16, tag=wname, name=f"{wname}_sb")
            nc.sync.dma_start(t[:], wap.rearrange("(k p) n -> p k n", p=128))
            w_sb[wname] = t

        # per head h: qhl = [q_hi; q_lo] stacked on partitions, with the hi
        # half at parts 0-63 for even h and 64-127 for odd h (so hi sits at
        # the head's native psum parity). khh = [k_hi_h; k_hi_h] duplicated.
        # klo per PAIR m: [k_lo_even (0-63); k_lo_odd (64-127)].
        qhl_sb = [proj_out.tile([128, SEQ], f16, tag=f"qhl{h}", name=f"qhl{h}") for h in range(HPC)]
        khh_sb = [proj_out.tile([128, SEQ], f16, tag=f"khh{h}", name=f"khh{h}") for h in range(HPC)]
        klo_sb = [proj_out.tile([128, SEQ], f16, tag=f"klo{m}", name=f"klo{m}") for m in range(2)]
        v_sb = [
            proj_out.tile([128, HPC, DH + 1], f16, tag=f"v{s}", name=f"v{s}")
            for s in range(ST_N)
        ]

        def emit_proj_group(sc):
            """Project q/k (hi+lo split) and v for seq chunk sc."""
            xh, xl = [], []
            for k in range(KT_N):
                th = xs_pool.tile([128, 512], f16, tag=f"xh{k}", name=f"xh{k}")
                nc.sync.dma_start(
                    th[:], xhi[k * 128 : (k + 1) * 128, sc * 512 : (sc + 1) * 512]
                )
                xh.append(th)
                tl = xs_pool.tile([128, 512], f16, tag=f"xl{k}", name=f"xl{k}")
                nc.scalar.dma_start(
                    tl[:], xlo[k * 128 : (k + 1) * 128, sc * 512 : (sc + 1) * 512]
                )
                xl.append(tl)
            sl = slice(sc * 512, (sc + 1) * 512)
            for kind, whi, wlo in (
                ("k", "wkhi", "wklo"),
                ("q", "wqhi", "wqlo"),
            ):
                for m in range(2):
                    he, ho = 2 * m, 2 * m + 1  # even head (parts 0-63), odd
                    pst = ps_sc.tile([128, 1024], f32, tag="ps", name="pproj")
                    n = 3 * KT_N
                    i = 0
                    for k in range(KT_N):
                        whi_ap = w_sb[whi][:, k, m * 128 : (m + 1) * 128]
                        wlo_ap = w_sb[wlo][:, k, m * 128 : (m + 1) * 128]
                        for wap, xap in ((whi_ap, xh[k]), (whi_ap, xl[k]),
                                         (wlo_ap, xh[k])):
                            nc.tensor.matmul(
                                pst[:, 0:512], wap, xap[:],
                                start=(i == 0), stop=(i == n - 1),
                            )
                            i += 1
                    if kind == "k":
                        # hi halves -> khh at native parity, dup via DMA;
                        # lo halves -> klo pair tile at native parity.
                        nc.scalar.copy(khh_sb[he][0:64, sl], pst[0:64, 0:512])
                        nc.scalar.copy(khh_sb[ho][64:128, sl], pst[64:128, 0:512])
                        nc.gpsimd.dma_start(
                            khh_sb[he][64:128, sl], khh_sb[he][0:64, sl]
                        )
                        nc.gpsimd.dma_start(
                            khh_sb[ho][0:64, sl], khh_sb[ho][64:128, sl]
                        )
                        nc.vector.tensor_sub(
                            klo_sb[m][0:64, sl], pst[0:64, 0:512],
                            khh_sb[he][0:64, sl],
                        )
                        nc.vector.tensor_sub(
                            klo_sb[m][64:128, sl], pst[64:128, 0:512],
                            khh_sb[ho][64:128, sl],
                        )
                    else:
                        # q: hi at native parity (direct copy), lo at the
                        # other parity (bounce through scr + DMA move).
                        scr = scr_pool.tile(
                            [128, 512], f16, tag=f"scrq{m}", name=f"scrq{m}"
                        )
                        nc.scalar.copy(qhl_sb[he][0:64, sl], pst[0:64, 0:512])
                        nc.vector.tensor_sub(
                            scr[0:64, :], pst[0:64, 0:512], qhl_sb[he][0:64, sl]
                        )
                        nc.gpsimd.dma_start(qhl_sb[he][64:128, sl], scr[0:64, :])
                        nc.scalar.copy(qhl_sb[ho][64:128, sl], pst[64:128, 0:512])
                        nc.vector.tensor_sub(
                            scr[64:128, :], pst[64:128, 0:512],
                            qhl_sb[ho][64:128, sl],
                        )
                        nc.gpsimd.dma_start(qhl_sb[ho][0:64, sl], scr[64:128, :])
            for j in range(4):
                st = 4 * sc + j
                psvt = ps_sc.tile([128, 1024], f32, tag="ps", name="pv")
                for k in range(KT_N):
                    nc.tensor.matmul(
                        psvt[:, 0:DO],
                        xh[k][:, j * 128 : (j + 1) * 128],
                        w_sb["wv"][:, k, :],
                        start=(k == 0),
                        stop=(k == KT_N - 1),
                    )
                nc.scalar.copy(
                    v_sb[st][:, :, 0:DH],
                    psvt[:, 0:DO].rearrange("p (h d) -> p h d", h=HPC),
                )
                nc.gpsimd.memset(v_sb[st][:, :, DH : DH + 1], 1.0)

        def emit_scores_qt(qt, pt_tiles):
            """Scores+softmax rows for q-tile qt, heads sequential.

            Per 512-col chunk two matmuls accumulate: slot A (K=128,
            [q_hi;q_lo] x [k_hi;k_hi] = (q_hi+q_lo)*k_hi) and slot B (K=64,
            q_hi x k_lo). Exp reads PSUM directly with global -max bias."""
            L = (qt + 1) * 128
            qcols = slice(qt * 128, (qt + 1) * 128)
            for h in range(HPC):
                subs = [(0, min(L, 1024))]
                if L > 1024:
                    subs.append((1024, L - 1024))
                mneg_p = stats.tile([128, 2], f32, tag=f"mnegp{h}", name=f"mnegp{h}")
                p_row = prow_pool.tile([128, SEQ], f16, tag=f"prow{h}", name=f"prow{h}")
                ps_tiles = []
                for si, (off, ls) in enumerate(subs):
                    ps = ps_sc.tile([128, 1024], f32, tag="ps", name="srow")
                    ps_tiles.append((ps, off, ls))
                    po_ = (h % 2) * 64
                    for c0 in range(0, ls, 512):
                        c1 = min(ls, c0 + 512)
                        nc.tensor.matmul(
                            ps[:, c0:c1],
                            qhl_sb[h][:, qcols],
                            khh_sb[h][:, off + c0 : off + c1],
                            start=True,
                            stop=False,
                        )
                        nc.tensor.matmul(
                            ps[:, c0:c1],
                            qhl_sb[h][po_ : po_ + 64, qcols],
                            klo_sb[h // 2][po_ : po_ + 64, off + c0 : off + c1],
                            start=False,
                            stop=True,
                        )
                    if off + ls == L:
                        nc.vector.tensor_add(
                            ps[:, ls - 128 : ls], ps[:, ls - 128 : ls], mask_sb[:]
                        )
                    nc.vector.reduce_max(
                        mneg_p[:, si : si + 1],
                        ps[:, 0:ls],
                        axis=mybir.AxisListType.X,
                        negate=True,
                    )
                if len(subs) == 2:
                    mneg = stats.tile([128, 1], f32, tag=f"mneg{h}", name=f"mneg{h}")
                    nc.vector.tensor_reduce(
                        mneg[:, 0:1],
                        mneg_p[:, 0:2],
                        axis=mybir.AxisListType.X,
                        op=mybir.AluOpType.min,
                    )
                    mneg_ap = mneg[:, 0:1]
                else:
                    mneg_ap = mneg_p[:, 0:1]
                for ps, off, ls in ps_tiles:
                    nc.scalar.activation(
                        p_row[:, off : off + ls],
                        ps[:, 0:ls],
                        mybir.ActivationFunctionType.Exp,
                        bias=mneg_ap,
                        scale=1.0,
                    )
                eng = nc.sync if h % 2 == 0 else nc.scalar
                eng.dma_start_transpose(
                    pt_tiles[h][
                        :, : qt + 1, (qt % 4) * 128 : (qt % 4) * 128 + 128
                    ],
                    p_row[:, :L],
                )

        def emit_pv_qc(qc, pt_tiles):
            for h in range(HPC):
                po = ps_o.tile([65, 512], f32, tag="po", name="po")
                kt_hi = qc * 4 + 3
                for kt in range(kt_hi + 1):
                    off = max(0, (kt - qc * 4)) * 128
                    nc.tensor.matmul(
                        po[:, off:512],
                        v_sb[kt][:, h, :],
                        pt_tiles[h][:, kt, off:512],
                        start=(kt == 0),
                        stop=(kt == kt_hi),
                    )
                ot = outp.tile([65, 512], f32, tag="ot", name="ot")
                if h % 2 == 0:
                    nc.scalar.copy(ot[:], po[:])
                else:
                    nc.vector.tensor_copy(ot[:], po[:])
                nc.gpsimd.dma_start(outT[h, :, qc * 512 : (qc + 1) * 512], ot[:])

        # emission order: proj(0), scores(q c0), proj(1), PV(0), scores(1),
        # proj(2), PV(1), scores(2), proj(3), PV(2), scores(3), PV(3)
        # so PV's wait-on-transpose overlaps the next proj chunk's matmuls.
        pt_by_qc = {}
        for sc in range(QC_N):
            emit_proj_group(sc)
            if sc >= 1:
                emit_pv_qc(sc - 1, pt_by_qc.pop(sc - 1))
            pt_tiles = {
                h: ptrow_pool.tile(
                    [128, ST_N, 512], f16, tag=f"pt{h % 2}", name=f"pt{h % 2}"
                )
                for h in range(HPC)
            }
            pt_by_qc[sc] = pt_tiles
            for qt in range(sc * 4, sc * 4 + 4):
                emit_scores_qt(qt, pt_tiles)
        emit_pv_qc(QC_N - 1, pt_by_qc.pop(QC_N - 1))


def _split_waits(nc):
    """This container's walrus accepts at most ONE sync-wait per instruction
    on several opcodes ("Too many sync wait commands"). Hoist excess waits
    into standalone InstEventSemaphore instructions on the same engine."""
    from concourse import mybir

    cap = 1
    n = 0
    for f in nc.m.functions:
        for bb in f.blocks:
            new = []
            for inst in list(bb.instructions):
                si = inst.sync_info
                waits = list(si.on_wait) if si is not None else []
                if len(waits) > cap:
                    for j, w in enumerate(waits[cap:]):
                        new.append(
                            mybir.InstEventSemaphore(
                                name=f"{inst.name}-w{j}",
                                engine=inst.engine,
                                ins=[],
                                outs=[],
                                sync_info=mybir.SyncInfo(on_wait=[w], on_update=[]),
                            )
                        )
                        n += 1
                    inst.sync_info = mybir.SyncInfo(
                        on_wait=waits[:cap], on_update=list(si.on_update)
                    )
                new.append(inst)
            bb.instructions = new
    return n


def _build_nc():
    import concourse.bass as bass
    import concourse.tile as tile
    from concourse import mybir

    f32 = mybir.dt.float32
    f16 = mybir.dt.float16
    nc = bass.Bass(
        "TRN2",
        target_bir_lowering=False,
        debug=False,
        num_devices=NCORES,
    )
    xhi = nc.dram_tensor("xhi", [DIN, SEQ], f16, kind="ExternalInput").ap()
    xlo = nc.dram_tensor("xlo", [DIN, SEQ], f16, kind="ExternalInput").ap()
    wqhi = nc.dram_tensor("wqhi", [DIN, DO], f16, kind="ExternalInput").ap()
    wqlo = nc.dram_tensor("wqlo", [DIN, DO], f16, kind="ExternalInput").ap()
    wkhi = nc.dram_tensor("wkhi", [DIN, DO], f16, kind="ExternalInput").ap()
    wklo = nc.dram_tensor("wklo", [DIN, DO], f16, kind="ExternalInput").ap()
    wv = nc.dram_tensor("wv", [DIN, DO], f16, kind="ExternalInput").ap()
    mask = nc.dram_tensor("mask", [128, 128], f32, kind="ExternalInput").ap()
    outT = nc.dram_tensor("outT", [HPC, DH + 1, SEQ], f32, kind="ExternalOutput").ap()

    with tile.TileContext(nc) as tc:
        _emit_core_kernel(
            tc, (outT,), (xhi, xlo, wqhi, wqlo, wkhi, wklo, wv, mask)
        )
    _split_waits(nc)
    return nc


def make_mask():
    m = np.zeros((128, 128), dtype=np.float32)
    q = np.arange(128)[:, None]
    k = np.arange(128)[None, :]
    m[k > q] = NEG
    return m


def _split16(a):
    hi = a.astype(np.float16)
    lo = (a - hi.astype(np.float32)).astype(np.float16)
    return hi, lo


def shard_inputs(x, W_q, W_k, W_v):
    x = np.asarray(x, dtype=np.float32)
    W_q = np.asarray(W_q, dtype=np.float32)
    W_k = np.asarray(W_k, dtype=np.float32)
    W_v = np.asarray(W_v, dtype=np.float32)
    mask = make_mask()
    scale = 1.0 / math.sqrt(DH)
    in_maps = []
    xh_b, xl_b = [], []
    for b in range(B):
        hi, lo = _split16(np.ascontiguousarray(x[b].T))
        xh_b.append(hi)
        xl_b.append(lo)
    for c in range(NCORES):
        b, g = divmod(c, NCORES // B)
        sl = slice(g * DO, (g + 1) * DO)
        wqh, wql = _split16(W_q[:, sl] * scale)
        wkh, wkl = _split16(W_k[:, sl])
        in_maps.append(
            {
                "xhi": xh_b[b],
                "xlo": xl_b[b],
                "wqhi": np.ascontiguousarray(wqh),
                "wqlo": np.ascontiguousarray(wql),
                "wkhi": np.ascontiguousarray(wkh),
                "wklo": np.ascontiguousarray(wkl),
                "wv": np.ascontiguousarray(W_v[:, sl].astype(np.float16)),
                "mask": mask,
            }
        )
    return in_maps


def assemble_output(results):
    out = np.zeros((B, SEQ, DIN), dtype=np.float32)
    for c in range(NCORES):
        b, g = divmod(c, NCORES // B)
        oT = results[c]["outT"]  # [HPC, 65, SEQ]
        for h in range(HPC):
            col = g * DO + h * DH
            out[b, :, col : col + DH] = (oT[h, :DH, :] / oT[h, DH : DH + 1, :]).T
    return out


def _install_axon_ntff_hook():
    """Provide antenv.axon_hooks (missing in this image) so trace=True works
    under axon. Mirrors trn_agent_boot.trn_boot._ntff_profile_via_ctypes."""
    import contextlib
    import ctypes
    import sys
    import types

    if "antenv.axon_hooks" in sys.modules:
        return True
    try:
        lib = ctypes.CDLL("/opt/axon/libaxon_pjrt.so")
    except OSError:
        return False
    if not hasattr(lib, "axon_start_nrt_profile"):
        return False
    lib.axon_start_nrt_profile.argtypes = [
        ctypes.POINTER(ctypes.c_int64),
        ctypes.c_size_t,
    ]
    lib.axon_start_nrt_profile.restype = ctypes.c_int64
    lib.axon_stop_nrt_profile.argtypes = [ctypes.c_char_p]
    lib.axon_stop_nrt_profile.restype = ctypes.c_int64

    @contextlib.contextmanager
    def _hook(output_dir, device_ids):
        import jax

        jax.devices()
        if device_ids:
            ids = (ctypes.c_int64 * len(device_ids))(*device_ids)
            rc = lib.axon_start_nrt_profile(ids, len(device_ids))
        else:
            rc = lib.axon_start_nrt_profile(None, 0)
        if rc != 0:
            raise RuntimeError(f"axon_start_nrt_profile rc={rc}")
        try:
            yield
        finally:
            n = lib.axon_stop_nrt_profile(str(output_dir).encode())
            print(f"ntff profile: {n} file(s) written to {output_dir}")

    mod = types.ModuleType("antenv.axon_hooks")
    holder = [_hook]
    mod.get_axon_ntff_profile_hook = lambda: holder[0]
    mod.set_axon_ntff_profile_hook = lambda h: holder.__setitem__(0, h)
    sys.modules["antenv.axon_hooks"] = mod
    import antenv

    antenv.axon_hooks = mod
    return True


def kernel(x, W_q, W_k, W_v):
    global LAST_RESULTS
    import os

    import concourse.bass_utils as bass_utils
    from concourse.bass_utils import run_bass_kernel_spmd

    if "nc" not in _CACHE:
        _CACHE["nc"] = _build_nc()
    nc = _CACHE["nc"]

    in_maps = shard_inputs(x, W_q, W_k, W_v)

    trace = bool(int(os.environ.get("MHA_TRACE", "0")))
    if trace:
        trace = _install_axon_ntff_hook()
        # avoid the fish-bucket artifact upload in this container
        bass_utils.upload_artifacts = lambda d: str(d)
    res = run_bass_kernel_spmd(
        nc, in_maps, core_ids=list(range(NCORES)), trace=trace
    )
    LAST_RESULTS = res
    return assemble_output(res.results)


# revision 18
# speedup vs baseline: 1.1535x; 1.0321x over previous
"""Causal multi-head attention (B=2, S=2048, D=1024, H=16, Dh=64) on 8 TRN2 cores.

Sharding: core c -> batch b=c//4, head-group g=c%4 (heads 4g..4g+3, d_out cols
g*256..(g+1)*256). Each core computes Q/K/V projections for its head group from
x[b] and runs causal attention for its 4 heads independently. No collectives.

All matmuls run in fp16 (1 cyc/col vs fp32's ~4.2). Precision on the logit
path (std ~1024, near-one-hot softmax => argmax flips dominate error) is kept
via hi/lo fp16 splits:
  proj:   x = xhi+xlo, W = Whi+Wlo (split on host); q = xhi*Whi + xlo*Whi +
          xhi*Wlo (3 passes, drops xlo*Wlo ~ 2^-23 rel).
  scores: q = qhi+qlo, k = khi+klo (split on device from proj psum);
          s = qhi*khi + qlo*khi + qhi*klo. K=64 slots for heads h,h+1 run
          CONCURRENTLY via PE row tiling tile_position=(0,0)/(64,0).
Softmax avoids holding full score rows in PSUM: each 512-col chunk is reduced
(chunk max, negated) and shifted (s - m_c) into a fp16 row in SBUF; after all
chunks the global max is combined and exp runs in-place on ACT with per-chunk
bias (m_c - m). V carries a ones column so PV's matmul also yields the softmax
denominator; host divides and assembles.
"""

import math

import numpy as np

B = 2
SEQ = 2048
DIN = 1024
H = 16
DH = 64
NCORES = 8
DO = 256  # d_out columns per core (4 heads)
HPC = 4  # heads per core
KT_N = DIN // 128  # 8 contraction tiles
ST_N = SEQ // 128  # 16 seq tiles
QC_N = SEQ // 512  # 4 q-chunks
NEG = -30000.0  # stays finite in f16 after (s - m_c) shift; exp -> 0

_CACHE = {}
LAST_RESULTS = None


def _emit_core_kernel(tc, outs, ins):
    from concourse import mybir

    nc = tc.nc
    f32 = mybir.dt.float32
    f16 = mybir.dt.float16
    (outT,) = outs  # [HPC, 65, SEQ] f32
    xhi, xlo, wqhi, wqlo, wkhi, wklo, wv, mask = ins

    from contextlib import ExitStack

    with ExitStack() as ctx:
        consts = ctx.enter_context(tc.tile_pool(name="consts", bufs=1))
        proj_out = ctx.enter_context(tc.tile_pool(name="proj_out", bufs=1))
        xs_pool = ctx.enter_context(tc.tile_pool(name="xs", bufs=1))
        scr_pool = ctx.enter_context(tc.tile_pool(name="scr", bufs=1))
        prow_pool = ctx.enter_context(tc.tile_pool(name="prow", bufs=1))
        ptrow_pool = ctx.enter_context(tc.tile_pool(name="ptrow", bufs=2))
        stats = ctx.enter_context(tc.tile_pool(name="stats", bufs=8))
        outp = ctx.enter_context(tc.tile_pool(name="outp", bufs=2))
        ps_sc = ctx.enter_context(
            tc.tile_pool(name="ps_sc", bufs=3, space="PSUM")
        )
        ps_o = ctx.enter_context(tc.tile_pool(name="ps_o", bufs=2, space="PSUM"))

        mask_sb = consts.tile([128, 128], f32, tag="mask", name="mask")
        nc.sync.dma_start(mask_sb[:], mask[:])
        w_sb = {}
        for wname, wap in (
            ("wqhi", wqhi),
            ("wqlo", wqlo),
            ("wkhi", wkhi),
            ("wklo", wklo),
            ("wv", wv),
        ):
            t = consts.tile([128, KT_N, DO], f16, tag=wname, name=f"{wname}_sb")
            nc.sync.dma_start(t[:], wap.rearrange("(k p) n -> p k n", p=128))
            w_sb[wname] = t

        # per head h: qhl = [q_hi; q_lo] stacked on partitions, with the hi
        # half at parts 0-63 for even h and 64-127 for odd h (so hi sits at
        # the head's native psum parity). khh = [k_hi_h; k_hi_h] duplicated.
        # klo per PAIR m: [k_lo_even (0-63); k_lo_odd (64-127)].
        qhl_sb = [proj_out.tile([128, SEQ], f16, tag=f"qhl{h}", name=f"qhl{h}") for h in range(HPC)]
        khh_sb = [proj_out.tile([128, SEQ], f16, tag=f"khh{h}", name=f"khh{h}") for h in range(HPC)]
        klo_sb = [proj_out.tile([128, SEQ], f16, tag=f"klo{m}", name=f"klo{m}") for m in range(2)]
        v_sb = [
            proj_out.tile([128, HPC, DH + 1], f16, tag=f"v{s}", name=f"v{s}")
            for s in range(ST_N)
        ]

        def emit_proj_group(sc):
            """Project q/k (hi+lo split) and v for seq chunk sc."""
            xh, xl = [], []
            for k in range(KT_N):
                th = xs_pool.tile([128, 512], f16, tag=f"xh{k}", name=f"xh{k}")
                nc.sync.dma_start(
                    th[:], xhi[k * 128 : (k + 1) * 128, sc * 512 : (sc + 1) * 512]
                )
                xh.append(th)
                tl = xs_pool.tile([128, 512], f16, tag=f"xl{k}", name=f"xl{k}")
                nc.scalar.dma_start(
                    tl[:], xlo[k * 128 : (k + 1) * 128, sc * 512 : (sc + 1) * 512]
                )
                xl.append(tl)
            sl = slice(sc * 512, (sc + 1) * 512)
            for kind, whi, wlo in (
                ("k", "wkhi", "wklo"),
                ("q", "wqhi", "wqlo"),
            ):
                for m in range(2):
                    he, ho = 2 * m, 2 * m + 1  # even head (parts 0-63), odd
                    pst = ps_sc.tile([128, 1024], f32, tag="ps", name="pproj")
                    n = 3 * KT_N
                    i = 0
                    for k in range(KT_N):
                        whi_ap = w_sb[whi][:, k, m * 128 : (m + 1) * 128]
                        wlo_ap = w_sb[wlo][:, k, m * 128 : (m + 1) * 128]
                        for wap, xap in ((whi_ap, xh[k]), (whi_ap, xl[k]),
                                         (wlo_ap, xh[k])):
                            nc.tensor.matmul(
                                pst[:, 0:512], wap, xap[:],
                                start=(i == 0), stop=(i == n - 1),
                            )
                            i += 1
                    if kind == "k":
                        # hi halves -> khh at native parity, dup via DMA;
                        # lo halves -> klo pair tile at native parity.
                        nc.scalar.copy(khh_sb[he][0:64, sl], pst[0:64, 0:512])
                        nc.scalar.copy(khh_sb[ho][64:128, sl], pst[64:128, 0:512])
                        nc.gpsimd.dma_start(
                            khh_sb[he][64:128, sl], khh_sb[he][0:64, sl]
                        )
                        nc.gpsimd.dma_start(
                            khh_sb[ho][0:64, sl], khh_sb[ho][64:128, sl]
                        )
                        nc.vector.tensor_sub(
                            klo_sb[m][0:64, sl], pst[0:64, 0:512],
                            khh_sb[he][0:64, sl],
                        )
                        nc.vector.tensor_sub(
                            klo_sb[m][64:128, sl], pst[64:128, 0:512],
                            khh_sb[ho][64:128, sl],
                        )
                    else:
                        # q: hi at native parity (direct copy), lo at the
                        # other parity (bounce through scr + DMA move).
                        scr = scr_pool.tile(
                            [128, 512], f16, tag=f"scrq{m}", name=f"scrq{m}"
                        )
                        nc.scalar.copy(qhl_sb[he][0:64, sl], pst[0:64, 0:512])
                        nc.vector.tensor_sub(
                            scr[0:64, :], pst[0:64, 0:512], qhl_sb[he][0:64, sl]
                        )
                        nc.gpsimd.dma_start(qhl_sb[he][64:128, sl], scr[0:64, :])
                        nc.scalar.copy(qhl_sb[ho][64:128, sl], pst[64:128, 0:512])
                        nc.vector.tensor_sub(
                            scr[64:128, :], pst[64:128, 0:512],
                            qhl_sb[ho][64:128, sl],
                        )
                        nc.gpsimd.dma_start(qhl_sb[ho][0:64, sl], scr[64:128, :])
            for j in range(4):
                st = 4 * sc + j
                psvt = ps_sc.tile([128, 1024], f32, tag="ps", name="pv")
                for k in range(KT_N):
                    nc.tensor.matmul(
                        psvt[:, 0:DO],
                        xh[k][:, j * 128 : (j + 1) * 128],
                        w_sb["wv"][:, k, :],
                        start=(k == 0),
                        stop=(k == KT_N - 1),
                    )
                nc.scalar.copy(
                    v_sb[st][:, :, 0:DH],
                    psvt[:, 0:DO].rearrange("p (h d) -> p h d", h=HPC),
                )
                nc.gpsimd.memset(v_sb[st][:, :, DH : DH + 1], 1.0)

        def emit_scores_qt(qt, pt_tiles):
            """Scores+softmax rows for q-tile qt, heads sequential.

            Per 512-col chunk two matmuls accumulate: slot A (K=128,
            [q_hi;q_lo] x [k_hi;k_hi] = (q_hi+q_lo)*k_hi) and slot B (K=64,
            q_hi x k_lo). Exp reads PSUM directly with global -max bias."""
            L = (qt + 1) * 128
            qcols = slice(qt * 128, (qt + 1) * 128)
            for h in range(HPC):
                subs = [(0, min(L, 1024))]
                if L > 1024:
                    subs.append((1024, L - 1024))
                mneg_p = stats.tile([128, 2], f32, tag=f"mnegp{h}", name=f"mnegp{h}")
                p_row = prow_pool.tile([128, SEQ], f16, tag=f"prow{h}", name=f"prow{h}")
                ps_tiles = []
                po_ = (h % 2) * 64
                chunks = []
                for si, (off, ls) in enumerate(subs):
                    ps = ps_sc.tile([128, 1024], f32, tag="ps", name="srow")
                    ps_tiles.append((ps, off, ls))
                    for c0 in range(0, ls, 512):
                        chunks.append((ps, off, c0, min(ls, c0 + 512)))
                for ps, off, c0, c1 in chunks:
                    nc.tensor.matmul(
                        ps[:, c0:c1],
                        qhl_sb[h][:, qcols],
                        khh_sb[h][:, off + c0 : off + c1],
                        start=True,
                        stop=False,
                        skip_group_check=True,
                    )
                for ps, off, c0, c1 in chunks:
                    nc.tensor.matmul(
                        ps[:, c0:c1],
                        qhl_sb[h][po_ : po_ + 64, qcols],
                        klo_sb[h // 2][po_ : po_ + 64, off + c0 : off + c1],
                        start=False,
                        stop=True,
                        skip_group_check=True,
                    )
                for si, (ps, off, ls) in enumerate(ps_tiles):
                    if off + ls == L:
                        nc.vector.tensor_add(
                            ps[:, ls - 128 : ls], ps[:, ls - 128 : ls], mask_sb[:]
                        )
                    nc.vector.reduce_max(
                        mneg_p[:, si : si + 1],
                        ps[:, 0:ls],
                        axis=mybir.AxisListType.X,
                        negate=True,
                    )
                if len(subs) == 2:
                    mneg = stats.tile([128, 1], f32, tag=f"mneg{h}", name=f"mneg{h}")
                    nc.vector.tensor_reduce(
                        mneg[:, 0:1],
                        mneg_p[:, 0:2],
                        axis=mybir.AxisListType.X,
                        op=mybir.AluOpType.min,
                    )
                    mneg_ap = mneg[:, 0:1]
                else:
                    mneg_ap = mneg_p[:, 0:1]
                for ps, off, ls in ps_tiles:
                    nc.scalar.activation(
                        p_row[:, off : off + ls],
                        ps[:, 0:ls],
                        mybir.ActivationFunctionType.Exp,
                        bias=mneg_ap,
                        scale=1.0,
                    )
                nc.sync.dma_start_transpose(
                    pt_tiles[h][
                        :, : qt + 1, (qt % 4) * 128 : (qt % 4) * 128 + 128
                    ],
                    p_row[:, :L],
                )

        def emit_pv_qc(qc, pt_tiles):
            for h in range(HPC):
                po = ps_o.tile([65, 512], f32, tag="po", name="po")
                kt_hi = qc * 4 + 3
                for kt in range(kt_hi + 1):
                    off = max(0, (kt - qc * 4)) * 128
                    nc.tensor.matmul(
                        po[:, off:512],
                        v_sb[kt][:, h, :],
                        pt_tiles[h][:, kt, off:512],
                        start=(kt == 0),
                        stop=(kt == kt_hi),
                    )
                ot = outp.tile([65, 512], f32, tag="ot", name="ot")
                nc.vector.tensor_copy(ot[:], po[:])
                nc.gpsimd.dma_start(outT[h, :, qc * 512 : (qc + 1) * 512], ot[:])

        # emission order: proj(0), scores(q c0), proj(1), PV(0), scores(1),
        # proj(2), PV(1), scores(2), proj(3), PV(2), scores(3), PV(3)
        # so PV's wait-on-transpose overlaps the next proj chunk's matmuls.
        pt_by_qc = {}
        for sc in range(QC_N):
            emit_proj_group(sc)
            if sc >= 1:
                emit_pv_qc(sc - 1, pt_by_qc.pop(sc - 1))
            pt_tiles = {
                h: ptrow_pool.tile(
                    [128, ST_N, 512], f16, tag=f"pt{h % 2}", name=f"pt{h % 2}"
                )
                for h in range(HPC)
            }
            pt_by_qc[sc] = pt_tiles
            for qt in range(sc * 4, sc * 4 + 4):
                emit_scores_qt(qt, pt_tiles)
        emit_pv_qc(QC_N - 1, pt_by_qc.pop(QC_N - 1))


def _split_waits(nc):
    """This container's walrus accepts at most ONE sync-wait per instruction
    on several opcodes ("Too many sync wait commands"). Hoist excess waits
    into standalone InstEventSemaphore instructions on the same engine."""
    from concourse import mybir

    cap = 1
    n = 0
    for f in nc.m.functions:
        for bb in f.blocks:
            new = []
            for inst in list(bb.instructions):
                si = inst.sync_info
                waits = list(si.on_wait) if si is not None else []
                if len(waits) > cap:
                    for j, w in enumerate(waits[cap:]):
                        new.append(
                            mybir.InstEventSemaphore(
                                name=f"{inst.name}-w{j}",
                                engine=inst.engine,
                                ins=[],
                                outs=[],
                                sync_info=mybir.SyncInfo(on_wait=[w], on_update=[]),
                            )
                        )
                        n += 1
                    inst.sync_info = mybir.SyncInfo(
                        on_wait=waits[:cap], on_update=list(si.on_update)
                    )
                new.append(inst)
            bb.instructions = new
    return n


def _build_nc():
    import concourse.bass as bass
    import concourse.tile as tile
    from concourse import mybir

    f32 = mybir.dt.float32
    f16 = mybir.dt.float16
    nc = bass.Bass(
        "TRN2",
        target_bir_lowering=False,
        debug=False,
        num_devices=NCORES,
    )
    xhi = nc.dram_tensor("xhi", [DIN, SEQ], f16, kind="ExternalInput").ap()
    xlo = nc.dram_tensor("xlo", [DIN, SEQ], f16, kind="ExternalInput").ap()
    wqhi = nc.dram_tensor("wqhi", [DIN, DO], f16, kind="ExternalInput").ap()
    wqlo = nc.dram_tensor("wqlo", [DIN, DO], f16, kind="ExternalInput").ap()
    wkhi = nc.dram_tensor("wkhi", [DIN, DO], f16, kind="ExternalInput").ap()
    wklo = nc.dram_tensor("wklo", [DIN, DO], f16, kind="ExternalInput").ap()
    wv = nc.dram_tensor("wv", [DIN, DO], f16, kind="ExternalInput").ap()
    mask = nc.dram_tensor("mask", [128, 128], f32, kind="ExternalInput").ap()
    outT = nc.dram_tensor("outT", [HPC, DH + 1, SEQ], f32, kind="ExternalOutput").ap()

    with tile.TileContext(nc) as tc:
        _emit_core_kernel(
            tc, (outT,), (xhi, xlo, wqhi, wqlo, wkhi, wklo, wv, mask)
        )
    _split_waits(nc)
    return nc


def make_mask():
    m = np.zeros((128, 128), dtype=np.float32)
    q = np.arange(128)[:, None]
    k = np.arange(128)[None, :]
    m[k > q] = NEG
    return m


def _split16(a):
    hi = a.astype(np.float16)
    lo = (a - hi.astype(np.float32)).astype(np.float16)
    return hi, lo


def shard_inputs(x, W_q, W_k, W_v):
    x = np.asarray(x, dtype=np.float32)
    W_q = np.asarray(W_q, dtype=np.float32)
    W_k = np.asarray(W_k, dtype=np.float32)
    W_v = np.asarray(W_v, dtype=np.float32)
    mask = make_mask()
    scale = 1.0 / math.sqrt(DH)
    in_maps = []
    xh_b, xl_b = [], []
    for b in range(B):
        hi, lo = _split16(np.ascontiguousarray(x[b].T))
        xh_b.append(hi)
        xl_b.append(lo)
    for c in range(NCORES):
        b, g = divmod(c, NCORES // B)
        sl = slice(g * DO, (g + 1) * DO)
        wqh, wql = _split16(W_q[:, sl] * scale)
        wkh, wkl = _split16(W_k[:, sl])
        in_maps.append(
            {
                "xhi": xh_b[b],
                "xlo": xl_b[b],
                "wqhi": np.ascontiguousarray(wqh),
                "wqlo": np.ascontiguousarray(wql),
                "wkhi": np.ascontiguousarray(wkh),
                "wklo": np.ascontiguousarray(wkl),
                "wv": np.ascontiguousarray(W_v[:, sl].astype(np.float16)),
                "mask": mask,
            }
        )
    return in_maps


def assemble_output(results):
    out = np.zeros((B, SEQ, DIN), dtype=np.float32)
    for c in range(NCORES):
        b, g = divmod(c, NCORES // B)
        oT = results[c]["outT"]  # [HPC, 65, SEQ]
        for h in range(HPC):
            col = g * DO + h * DH
            out[b, :, col : col + DH] = (oT[h, :DH, :] / oT[h, DH : DH + 1, :]).T
    return out


def _install_axon_ntff_hook():
    """Provide antenv.axon_hooks (missing in this image) so trace=True works
    under axon. Mirrors trn_agent_boot.trn_boot._ntff_profile_via_ctypes."""
    import contextlib
    import ctypes
    import sys
    import types

    if "antenv.axon_hooks" in sys.modules:
        return True
    try:
        lib = ctypes.CDLL("/opt/axon/libaxon_pjrt.so")
    except OSError:
        return False
    if not hasattr(lib, "axon_start_nrt_profile"):
        return False
    lib.axon_start_nrt_profile.argtypes = [
        ctypes.POINTER(ctypes.c_int64),
        ctypes.c_size_t,
    ]
    lib.axon_start_nrt_profile.restype = ctypes.c_int64
    lib.axon_stop_nrt_profile.argtypes = [ctypes.c_char_p]
    lib.axon_stop_nrt_profile.restype = ctypes.c_int64

    @contextlib.contextmanager
    def _hook(output_dir, device_ids):
        import jax

        jax.devices()
        if device_ids:
            ids = (ctypes.c_int64 * len(device_ids))(*device_ids)
            rc = lib.axon_start_nrt_profile(ids, len(device_ids))
        else:
            rc = lib.axon_start_nrt_profile(None, 0)
        if rc != 0:
            raise RuntimeError(f"axon_start_nrt_profile rc={rc}")
        try:
            yield
        finally:
            n = lib.axon_stop_nrt_profile(str(output_dir).encode())
            print(f"ntff profile: {n} file(s) written to {output_dir}")

    mod = types.ModuleType("antenv.axon_hooks")
    holder = [_hook]
    mod.get_axon_ntff_profile_hook = lambda: holder[0]
    mod.set_axon_ntff_profile_hook = lambda h: holder.__setitem__(0, h)
    sys.modules["antenv.axon_hooks"] = mod
    import antenv

    antenv.axon_hooks = mod
    return True


def kernel(x, W_q, W_k, W_v):
    global LAST_RESULTS
    import os

    import concourse.bass_utils as bass_utils
    from concourse.bass_utils import run_bass_kernel_spmd

    if "nc" not in _CACHE:
        _CACHE["nc"] = _build_nc()
    nc = _CACHE["nc"]

    in_maps = shard_inputs(x, W_q, W_k, W_v)

    trace = bool(int(os.environ.get("MHA_TRACE", "0")))
    if trace:
        trace = _install_axon_ntff_hook()
        # avoid the fish-bucket artifact upload in this container
        bass_utils.upload_artifacts = lambda d: str(d)
    res = run_bass_kernel_spmd(
        nc, in_maps, core_ids=list(range(NCORES)), trace=trace
    )
    LAST_RESULTS = res
    return assemble_output(res.results)
